# revision 32
# baseline (speedup 1.0000x reference)
"""Multi-head causal self-attention (B=4, T=2048, C=768, H=12) on 8 trn2 cores.

Sharding: core c handles batch b = c//2 and head-group hg = c%2 (6 heads each).
Each core computes its QKV projection slice, causal attention for its 6 heads,
and a partial output projection (768x2048, transposed). Host sums the two
partials per batch, transposes back, and adds b_o. No cross-core collectives.

Key speed structure vs the fp32r baseline:
- all inputs are pre-transposed (and pre-quantized to fp8e4m3 where used as
  fp8) on the HOST, so the kernel does zero on-chip input transposes;
- QKV projection and the PV matmul run as fp8 DoubleRow matmuls (2 k-tiles
  of 128 contracted per pass at 0.5 cycles/row) except where softmax rows
  have too few summands to average out fp8 noise: rows q < 512 (and the
  t < 512 slice of QKV) stay fp32r, keeping rel err ~3e-3;
- V is produced in natural [t, d] layout directly by the projection (no V
  transposes); softmax denominators come from an appended ones column;
- exp on the ACT engine writes fp8 att tiles already in the DoubleRow
  [128, 2, cols] rhs layout; causal masking is applied pre-exp in PSUM by
  gpsimd affine_select with a -1e5 fill;
- the attention stream is ordered qt-major (q-chunk of 512 across all heads)
  so output-projection chunks of earlier qt overlap later attention instead
  of forming a serial tail; QKV chunk tt feeds attention block qt=tt, which
  only needs K/V up to (qt+1)*512 (causality).
"""

import math
import os

import numpy as np
import ml_dtypes

import concourse.bass as bass
from concourse import bacc
import concourse.mybir as mybir
import concourse.tile as tile
from concourse import bass_utils
from concourse.bass import ts
from concourse.masks import make_identity

F32 = mybir.dt.float32
F32R = mybir.dt.float32r
F8 = mybir.dt.float8e4
BF16 = mybir.dt.bfloat16
DR = mybir.MatmulPerfMode.DoubleRow

P = 128
T = 2048          # sequence length
C = 768           # embed dim
CS = C // P       # 6 contraction chunks
HL = 6            # heads per core
HD = 64           # head dim
J = HL * HD       # 384 local y-feature dim
JS = J // P       # 3
O = 3 * J         # 1152 rows of the local W_attn slice (q|k|v)
OB = O // P       # 9
QKOB = 6          # q,k row blocks
OUTB = C // P     # 6 output row blocks
NQT = 4           # 512-col q chunks
NPAIR = 8         # 256-row k pairs
HDP = 72          # padded head stride in vaug (dual-fp8 needs 16B-aligned steps)
SCALE = 1.0 / math.sqrt(HD)


def _build_bass():
    nc = bacc.Bacc("TRN2", target_bir_lowering=False, debug=False)
    xt32_d = nc.dram_tensor("xt32", [C, 512], BF16, kind="ExternalInput").ap()
    xt8_d = nc.dram_tensor("xt8", [C, T], F8, kind="ExternalInput").ap()
    wt32_d = nc.dram_tensor("wt32", [C, O], BF16, kind="ExternalInput").ap()
    wt8_d = nc.dram_tensor("wt8", [C, O], F8, kind="ExternalInput").ap()
    wot_d = nc.dram_tensor("wot", [J, C], F32R, kind="ExternalInput").ap()
    bqk_d = nc.dram_tensor("bqk", [2 * J], F32, kind="ExternalInput").ap()
    bvf_d = nc.dram_tensor("bvf", [P, J], F32, kind="ExternalInput").ap()
    out_d = nc.dram_tensor("out", [C, T], F32, kind="ExternalOutput").ap()

    with tile.TileContext(nc) as tc, nc.allow_low_precision(
        reason="fp8 doublerow + fp32r pipeline; fp32 PSUM accumulation"
    ):
        _emit_kernel(tc, xt32_d, xt8_d, wt32_d, wt8_d, wot_d, bqk_d, bvf_d, out_d)
    nc.compile()
    return nc


def _emit_kernel(tc, xt32_d, xt8_d, wt32_d, wt8_d, wot_d, bqk_d, bvf_d, out_d):
    nc = tc.nc

    xt32_r = xt32_d.rearrange("(cs p) t -> p cs t", p=P)   # [128, 6, 512]
    xt8_r = xt8_d.rearrange("(cs p) t -> p cs t", p=P)     # [128, 6, 2048]
    wt32_r = wt32_d.rearrange("(cs p) o -> p cs o", p=P)   # [128, 6, 1152]
    wt8_r = wt8_d.rearrange("(cs p) o -> p cs o", p=P)     # [128, 6, 1152]
    wot_r = wot_d.rearrange("(jb p) o -> p jb o", p=P)     # [128, 3, 768]
    bqk_r = bqk_d.rearrange("(a p) -> p a", p=P)           # [128, 6]
    out_r = out_d.rearrange("(ob p) t -> p ob t", p=P)     # [128, 6, 2048]

    with (
        tc.tile_pool(name="persist", bufs=1) as persist,
        tc.tile_pool(name="att", bufs=6) as attp,
        tc.tile_pool(name="att32", bufs=4) as attp32,
        tc.tile_pool(name="small", bufs=3) as small,
        tc.tile_pool(name="stage", bufs=3) as stage,
        tc.tile_pool(name="oacc", bufs=6) as oaccp,
        tc.tile_pool(name="otail", bufs=1) as otailp,
        tc.tile_pool(name="ps_sp", bufs=2, space="PSUM") as ps_sp,
        tc.tile_pool(name="ps_ya", bufs=2, space="PSUM") as ps_ya,
        tc.tile_pool(name="ps_mm", bufs=2, space="PSUM") as ps_mm,
    ):
        # ---- persistent SBUF tensors
        xt32 = persist.tile([P, CS, 512], BF16)     # 6KB/part
        xt8 = persist.tile([P, CS, T], F8)          # 12KB
        wt32 = persist.tile([P, CS, O], BF16)       # 13.5KB
        wt8 = persist.tile([P, CS, O], F8)          # 6.75KB
        wot = persist.tile([P, JS, C], F32R)        # 9KB
        bqk = persist.tile([P, QKOB], F32)
        bvf = persist.tile([P, J], F32)
        qkT = persist.tile([P, QKOB, T], F32R)      # 48KB  (q ob 0-2, k ob 3-5)
        yT = persist.tile([P, JS, T], F32R)         # 24KB
        vaug8 = persist.tile([P, NPAIR, 2, HL, HDP], F8)      # 6.75KB
        vaug32 = persist.tile([P, 2, 2, HL, HDP], F32R)       # 6.75KB (k<512)
        onesf = small.tile([P, HD], F32, tag="init", name="onesf")
        nc.vector.memset(onesf, 1.0)
        identf = small.tile([P, P], F32, tag="init", name="identf")
        make_identity(nc, identf)
        identr = persist.tile([P, P], F32R)
        nc.vector.tensor_copy(identr, identf)
        # ones columns of vaug (fp8 1.0 and f32 1.0)
        ones2h = onesf[:, 0 : 2 * HL].rearrange("p (a b) -> p a b", b=HL)
        for pair in range(NPAIR):
            nc.vector.tensor_copy(vaug8[:, pair, :, :, HD], ones2h)
        for pair in range(2):
            nc.vector.tensor_copy(vaug32[:, pair, :, :, HD], ones2h)

        # ---- input DMAs (halved for queue parallelism)
        def dma2(dst, src, axis_len):
            h = axis_len // 2
            nc.sync.dma_start(dst[..., :h], src[..., :h])
            nc.sync.dma_start(dst[..., h:], src[..., h:])

        nc.sync.dma_start(bqk, bqk_r)
        nc.sync.dma_start(bvf, bvf_d)
        nc.sync.dma_start(xt32, xt32_r)
        nc.sync.dma_start(wt32[:, :, 0:P], wt32_r[:, :, 0:P])
        nc.sync.dma_start(wt32[:, :, 3 * P : 4 * P], wt32_r[:, :, 3 * P : 4 * P])
        nc.sync.dma_start(wt32[:, :, P : 3 * P], wt32_r[:, :, P : 3 * P])
        nc.sync.dma_start(wt32[:, :, 4 * P : 2 * J], wt32_r[:, :, 4 * P : 2 * J])
        nc.sync.dma_start(wt32[:, :, 2 * J :], wt32_r[:, :, 2 * J :])
        dma2(wt8, wt8_r, O)
        dma2(xt8, xt8_r, T)
        dma2(wot, wot_r, C)

        # PE p-state warmup: ~10 small matmuls on the ones tile keep the
        # tensor engine continuously busy through the input-DMA wait so real
        # work starts at full clock instead of mid-ramp.
        pwarm = ps_mm.tile([P, 512], F32, tag="mm", name="pwarm")
        for _ in range(20):
            nc.tensor.matmul(
                pwarm[0:HD, 0:HD], onesf, onesf, start=True, stop=True
            )

        # ================= building blocks =================

        def emit_qkv32(ob):
            # qkT[:, ob, 0:512] for q/k section ob (0..5), bf16, t < 512
            pq = ps_mm.tile([P, 512], F32, tag="mm")
            for cs in range(CS):
                nc.tensor.matmul(
                    pq, wt32[:, cs, ts(ob, P)], xt32[:, cs, :],
                    start=(cs == 0), stop=(cs == CS - 1),
                )
            nc.vector.tensor_scalar_add(qkT[:, ob, 0:512], pq, bqk[:, ob : ob + 1])

        def emit_qkv8(ob, tt):
            # qkT[:, ob, tt*512:+512] fp8 DoubleRow, tt in 1..3
            pq = ps_mm.tile([P, 512], F32, tag="mm")
            for i in range(3):
                nc.tensor.matmul(
                    pq,
                    wt8[:, 2 * i : 2 * i + 2, ts(ob, P)],
                    xt8[:, 2 * i : 2 * i + 2, ts(tt, 512)],
                    start=(i == 0), stop=(i == 2), perf_mode=DR,
                )
            nc.vector.tensor_scalar_add(
                qkT[:, ob, ts(tt, 512)], pq, bqk[:, ob : ob + 1]
            )

        def emit_v32(tb):
            # natural-layout v for t-block tb (0..3), fp32r -> vaug32 AND vaug8
            pvt = ps_mm.tile([P, 512], F32, tag="mm", name="pvt")
            pv = pvt[:, 0:J]
            for cs in range(CS):
                nc.tensor.matmul(
                    pv, xt32[:, cs, ts(tb, P)], wt32[:, cs, 2 * J : 3 * J],
                    start=(cs == 0), stop=(cs == CS - 1),
                )
            pair, i = divmod(tb, 2)
            dst32 = vaug32[:, pair, i, :, 0:HD]
            dst8 = vaug8[:, pair, i, :, 0:HD]
            nc.vector.tensor_add(dst32, pv.rearrange("p (h d) -> p h d", d=HD),
                                 bvf.rearrange("p (h d) -> p h d", d=HD))
            nc.vector.tensor_add(dst8, pv.rearrange("p (h d) -> p h d", d=HD),
                                 bvf.rearrange("p (h d) -> p h d", d=HD))

        def emit_v8(tb):
            # natural-layout v for t-block tb (4..15), fp8 DoubleRow -> vaug8
            pvt = ps_mm.tile([P, 512], F32, tag="mm", name="pvt")
            pv = pvt[:, 0:J]
            for i in range(3):
                nc.tensor.matmul(
                    pv,
                    xt8[:, 2 * i : 2 * i + 2, ts(tb, P)],
                    wt8[:, 2 * i : 2 * i + 2, 2 * J : 3 * J],
                    start=(i == 0), stop=(i == 2), perf_mode=DR,
                )
            pair, i = divmod(tb, 2)
            nc.vector.tensor_add(
                vaug8[:, pair, i, :, 0:HD],
                pv.rearrange("p (h d) -> p h d", d=HD),
                bvf.rearrange("p (h d) -> p h d", d=HD),
            )

        def emit_outproj(tt):
            # part^T[o, tt*512:+512] = sum_j wot[j, o] yT[j, t]
            for ob in range(OUTB):
                po = ps_mm.tile([P, 512], F32, tag="mm")
                for js in range(JS):
                    nc.tensor.matmul(
                        po, wot[:, js, ts(ob, P)], yT[:, js, ts(tt, 512)],
                        start=(js == 0), stop=(js == JS - 1),
                    )
                osb = stage.tile([P, 512], F32, tag="osb")
                nc.vector.tensor_copy(osb, po)
                nc.sync.dma_start(out_r[:, ob, ts(tt, 512)], osb)

        oacc_tiles = {}

        def emit_outproj_p1(tt, ob):
            # heads 0-3 contribution (jb 0,1) -> SBUF accumulator
            po = ps_mm.tile([P, 512], F32, tag="mm")
            for js in range(JS - 1):
                nc.tensor.matmul(
                    po, wot[:, js, ts(ob, P)], yT[:, js, ts(tt, 512)],
                    start=(js == 0), stop=(js == JS - 2),
                )
            oa = oaccp.tile([P, 512], F32R, tag="oacc", name=f"oa{ob}")
            nc.vector.tensor_copy(oa, po)
            oacc_tiles[ob] = oa

        otail = otailp.tile([P, OUTB, 512], F32)

        def emit_outproj_p2(tt, ob):
            # heads 4,5 (jb 2) + identity-accumulate of the p1 partial (PE),
            # then PSUM->SBUF copy on alternating DVE/ACT
            po = ps_mm.tile([P, 512], F32, tag="mm")
            nc.tensor.matmul(
                po, wot[:, JS - 1, ts(ob, P)], yT[:, JS - 1, ts(tt, 512)],
                start=True, stop=False,
            )
            nc.tensor.matmul(
                po, identr, oacc_tiles[ob], start=False, stop=True,
            )
            if ob % 2 == 0:
                nc.vector.tensor_copy(otail[:, ob, :], po)
            else:
                nc.scalar.copy(otail[:, ob, :], po)
            nc.sync.dma_start(out_r[:, ob, ts(tt, 512)], otail[:, ob, :])

        # ================= attention =================
        # unit (h, qt, p): q cols [q0, (qt+1)*512), k pair p (256 rows)

        def unit_geometry(qt, p):
            q0 = max(p * 256, qt * 512)
            cols = (qt + 1) * 512 - q0
            rel = q0 - qt * 512          # 0 or 256
            diag = q0 == p * 256
            return q0, cols, rel, diag

        def emit_scores_exp(h, qt, p):
            """scores (PE) + mask (Pool) + exp (ACT) -> att tile for the unit."""
            q0, cols, rel, diag = unit_geometry(qt, p)
            p0 = (h % 2) * HD
            qTs = qkT[p0 : p0 + HD, h // 2, :]
            kTs = qkT[p0 : p0 + HD, 3 + h // 2, :]
            sp = ps_sp.tile([P, 2, 512], F32, tag="sp")
            # block A (k rows 2p*128..+128): valid from q >= 2p*128 <= q0
            nc.tensor.matmul(
                sp[:, 0, 0:cols], kTs[:, ts(2 * p, P)], qTs[:, q0 : q0 + cols],
                start=True, stop=True,
            )
            # block B: valid from q >= (2p+1)*128; on diagonal units the wedge
            # [0,128) holds finite wrong-side scores, zeroed post-exp below
            nc.tensor.matmul(
                sp[:, 1, 0:cols],
                kTs[:, ts(2 * p + 1, P)], qTs[:, q0 : q0 + cols],
                start=True, stop=True,
            )
            if qt == 0:
                att = attp32.tile([P, 2, 512], F32R, tag="att32")
            else:
                att = attp.tile([P, 2, 512], F8, tag="att")
            nc.scalar.activation(
                att[:, :, 0:cols], sp[:, :, 0:cols],
                mybir.ActivationFunctionType.Exp, scale=SCALE,
            )
            if diag:
                nc.gpsimd.affine_select(
                    out=att[:, :, 0 : 2 * P], in_=att[:, :, 0 : 2 * P],
                    compare_op=mybir.AluOpType.is_ge,
                    fill=0.0, base=0, channel_multiplier=-1,
                    pattern=[[-P, 2], [1, 2 * P]],
                )
            return att

        def emit_pv(h, qt, p, att, ya):
            q0, cols, rel, diag = unit_geometry(qt, p)
            start = p == 2 * qt + 1
            stop = p == 0
            if qt == 0:
                for i in range(2):
                    nc.tensor.matmul(
                        ya[0 : HD + 1, rel : rel + cols],
                        vaug32[:, p, i, h, 0 : HD + 1],
                        att[:, i, 0:cols],
                        start=(start and i == 0), stop=(stop and i == 1),
                    )
            else:
                nc.tensor.matmul(
                    ya[0 : HD + 1, rel : rel + cols],
                    vaug8[:, p, :, h, 0 : HD + 1],
                    att[:, :, 0:cols],
                    start=start, stop=stop, perf_mode=DR,
                )

        norm_q = []

        def flush_norms():
            while norm_q:
                h, qt, ya = norm_q.pop(0)
                p0 = (h % 2) * HD
                rd = small.tile([1, 512], F32R, tag="rd")
                nc.vector.reciprocal(rd, ya[HD : HD + 1, :])
                bcs = small.tile([HD, 512], F32R, tag="bcs")
                nc.gpsimd.partition_broadcast(bcs, rd)
                nc.vector.tensor_mul(
                    out=yT[p0 : p0 + HD, h // 2, ts(qt, 512)],
                    in0=ya[0:HD], in1=bcs,
                )

        # ================= schedule =================
        fillers = []   # (need_qt, fn): must run before attn block need_qt

        def pump(n=1):
            while n > 0 and fillers:
                tag, fn = fillers.pop(0)
                if fn is None:
                    key = tag[1]
                    fn2 = filler_fns.pop(key, None)
                    if fn2 is None:
                        continue
                    fn2()
                else:
                    fn()
                n -= 1

        def drain(up_to_qt):
            while fillers and (
                fillers[0][0] if not isinstance(fillers[0][0], tuple)
                else fillers[0][0][0]
            ) <= up_to_qt:
                tag, fn = fillers.pop(0)
                if fn is None:
                    fn2 = filler_fns.pop(tag[1], None)
                    if fn2 is not None:
                        fn2()
                else:
                    fn()

        # qt0 prerequisites emitted directly (q,k,v for t<512)
        for ob in (0, 3):
            emit_qkv32(ob)
        head_ready = [(1, 4), (2, 5)]  # qkv32 obs to emit before heads 2/4

        # fp8 fillers for later qt blocks, keyed for directed gating
        filler_fns = {}
        for tt in range(1, NQT):
            for ob in range(QKOB):
                filler_fns[("qkv8", ob, tt)] = lambda ob=ob, tt=tt: emit_qkv8(ob, tt)
            for tb in range(4 * tt, 4 * tt + 4):
                filler_fns[("v8", tb)] = lambda tb=tb: emit_v8(tb)
        for tt in range(1, NQT):
            for ob in range(QKOB):
                fillers.append(((tt, ("qkv8", ob, tt)), None))
            for tb in range(4 * tt, 4 * tt + 4):
                fillers.append(((tt, ("v8", tb)), None))

        def force_emit(keys):
            for k in keys:
                fn = filler_fns.pop(k, None)
                if fn is not None:
                    fn()

        pend = []   # deferred PV units: (h, qt, p, att, ya, last)

        def pop_unit():
            h, qt, p, att, ya, last = pend.pop(0)
            emit_pv(h, qt, p, att, ya)
            if last:
                norm_q.append((h, qt, ya))

        n_attn_units = 0
        block_seq = [(qt, h) for qt in range(NQT) for h in range(HL)]
        drained = set()
        done_count = {qt: 0 for qt in range(NQT)}
        for qt, h in block_seq:
            if qt not in drained:
                drained.add(qt)
                drain(qt)
            if qt == 1:
                force_emit([("qkv8", h // 2, 1), ("qkv8", 3 + h // 2, 1)]
                           + [("v8", tb) for tb in range(4, 8)])
            if qt == 0 and h == 1:
                emit_v32(2)
                emit_v32(3)
                emit_v32(0)
                emit_v32(1)
            if qt == 0 and h in (2, 4):
                for ob in head_ready[h // 2 - 1]:
                    emit_qkv32(ob)
            if qt == NQT - 1 and h == HL - 1:
                for ob in range(OUTB):
                    fillers.append((NQT, lambda ob=ob: emit_outproj_p1(3, ob)))
            ya = ps_ya.tile([P, 512], F32, tag="ya", name=f"ya{h}_{qt}")
            for p in reversed(range(2 * qt + 2)):
                flush_norms()
                att = emit_scores_exp(h, qt, p)
                pend.append((h, qt, p, att, ya, p == 0))
                while len(pend) > 4:
                    pop_unit()
                n_attn_units += 1
                # don't pull fp8-dependent fillers into the PE stream
                # before their DMAs have landed (~early qt0)
                if n_attn_units > 6:
                    pump(1)
            done_count[qt] += 1
            if done_count[qt] == HL:
                while pend:
                    pop_unit()
                flush_norms()
                if qt < NQT - 1:
                    fillers.append((NQT, lambda tt=qt: emit_outproj(tt)))
        drain(NQT)
        flush_norms()
        for ob in range(OUTB):
            emit_outproj_p2(3, ob)


_NC_CACHE = None
LAST_RESULTS = None


def _get_nc():
    global _NC_CACHE
    if _NC_CACHE is None:
        _NC_CACHE = _build_bass()
    return _NC_CACHE


def kernel(x, W_attn, b_attn, W_o, b_o):
    global LAST_RESULTS
    x = np.asarray(x, np.float32)
    W_attn = np.asarray(W_attn, np.float32)
    b_attn = np.asarray(b_attn, np.float32)
    W_o = np.asarray(W_o, np.float32)
    b_o = np.asarray(b_o, np.float32)
    F8NP = ml_dtypes.float8_e4m3

    B = x.shape[0]
    in_maps = []
    for core in range(8):
        b, hg = divmod(core, 2)
        sl = slice(hg * J, (hg + 1) * J)
        w_l = np.concatenate(
            [W_attn[sl], W_attn[C + hg * J : C + (hg + 1) * J],
             W_attn[2 * C + hg * J : 2 * C + (hg + 1) * J]], axis=0
        )  # [1152, 768]
        b_l = np.concatenate(
            [b_attn[sl], b_attn[C + hg * J : C + (hg + 1) * J],
             b_attn[2 * C + hg * J : 2 * C + (hg + 1) * J]], axis=0
        )  # [1152]
        xt = np.ascontiguousarray(x[b].T)              # [768, 2048]
        wt = np.ascontiguousarray(w_l.T)               # [768, 1152]
        in_maps.append({
            "xt32": np.ascontiguousarray(xt[:, :512]).astype(ml_dtypes.bfloat16),
            "xt8": xt.astype(F8NP),
            "wt32": wt.astype(ml_dtypes.bfloat16),
            "wt8": wt.astype(F8NP),
            "wot": np.ascontiguousarray(W_o[:, sl].T),  # [384, 768]
            "bqk": np.ascontiguousarray(b_l[: 2 * J]),
            "bvf": np.broadcast_to(b_l[2 * J :], (P, J)).copy(),
        })

    nc = _get_nc()
    LAST_RESULTS = bass_utils.run_bass_kernel_spmd(
        nc, in_maps, core_ids=list(range(8)),
        trace=bool(int(os.environ.get("KERNEL_TRACE", "0"))),
    )
    parts = [r["out"] for r in LAST_RESULTS.results]

    out = np.empty((B, T, C), np.float32)
    for b in range(B):
        out[b] = (parts[2 * b] + parts[2 * b + 1]).T + b_o
    return out
